# revision 24
# baseline (speedup 1.0000x reference)
"""Trainium2 Bass kernel for nn_DGASEncoder (PointNet++-style ball-query encoder).

Self-contained: hardcoded shapes; takes FULL inputs, shards across 8 NeuronCores
(data-parallel over (batch, N/2)), returns the FULL output.

Per-core pipeline (SPMD; cores differ only in input data):
  A) conv1d+BN stats: h = W1 @ f (bf16) over all B*N via PE; bn_stats/bn_aggr ->
     mean/var; h_n = relu(gamma'*h + beta') f32; swizzled fp16 transpose hT in
     SBUF (token n = partition n%128, stripe n//128).
  B) ball query per 128-query tile: squared distances via 24-row bf16-split
     matmuls (f32-exact); indicator on ACT/DVE; per-chunk valid counts via
     one-hot-column stationary matmuls; ranks via per-chunk triangular matmuls
     + carry matmuls; slot ids evacuated as int16; first-K extraction with
     gpsimd local_scatter (lib 7).
  C) neighbor features via SWDGE dma_gather (lib 3) straight out of SBUF hT
     (no DRAM round-trip); fused add with bf16 dlp stream + max-pool over K on
     DVE (fp16, 2x mode). All scatters are forced before the first gather so
     the gpsimd IRAM library is switched exactly once.
"""
import functools
import numpy as np
import ml_dtypes

B, N, C, K = 4, 4096, 128, 32
RADIUS = 0.1
BN_EPS = 1e-5
R2 = RADIUS * RADIUS
NCHUNK = 128
NCH = N // NCHUNK          # 32
QPC = N // 2               # 2048 queries per core
QT = 128                   # queries per tile
NQT = QPC // QT            # 16
QB = 512                   # query block
NQB = QPC // QB            # 4
GRP = 4                    # chunks per rank-psum group
NGRP = NCH // GRP          # 8
NCORES = 8
GC = 512                   # gather chunk (idxs per dma_gather)

bf16 = ml_dtypes.bfloat16


def _bf(x):
    return np.asarray(x, dtype=bf16).astype(np.float32)


def _split3(x):
    h = _bf(x)
    m = _bf(x - h)
    l = _bf(x - h - m)
    return h, m, l


def _build_AB(p_b, q_lo, q_hi):
    """A [24, N] (candidate side, -2 scale folded), Bm [24, Q] (query side), bf16."""
    x = p_b.astype(np.float32)
    pn = np.sum(x * x, axis=1, dtype=np.float32)
    sp = [_split3(x[:, d]) for d in range(3)]
    sp2 = [tuple(-2.0 * t for t in sp[d]) for d in range(3)]
    spn = _split3(pn)
    PAIRS = [(0, 0), (0, 1), (1, 0), (0, 2), (2, 0), (1, 1)]
    A_rows, B_rows = [], []
    for d in range(3):
        for (s, s2) in PAIRS:
            A_rows.append(sp2[d][s])
            B_rows.append(sp[d][s2][q_lo:q_hi])
    for s in range(3):
        A_rows.append(spn[s])
        B_rows.append(np.ones(q_hi - q_lo, np.float32))
    for s in range(3):
        A_rows.append(np.ones(N, np.float32))
        B_rows.append(spn[s][q_lo:q_hi])
    return (np.stack(A_rows).astype(bf16), np.stack(B_rows).astype(bf16))


def _static_tables():
    f8 = ml_dtypes.float8_e4m3fn
    # tri weights for v in {0,1}: slot contribution 32*v_t - (#valid before t)
    tri = np.zeros((NCHUNK, NCHUNK), np.float32)
    for jl in range(NCHUNK):
        tri[:jl, jl] = -1.0
        tri[jl, jl] = 32.0
    # DoubleRow moving for rank pairs: X0 = [tri | 0], X1 = [0 | tri]
    # packed two-major as [128, 512]
    tri_dr = np.zeros((NCHUNK, 2, 2 * NCHUNK), np.float32)
    tri_dr[:, 0, :NCHUNK] = tri
    tri_dr[:, 1, NCHUNK:] = tri
    tri_dr = tri_dr.reshape(NCHUNK, 4 * NCHUNK).astype(f8)
    # carry table [64, NCH*128]: rows 0..31 block-diag ones (ct3 hi), rows
    # 32..63 block-diag ones (ct3 lo); ct3 = -CNT_<c - 1.
    carry_all = np.zeros((64, NCH * NCHUNK), np.float32)
    for c in range(NCH):
        carry_all[c, c * NCHUNK:(c + 1) * NCHUNK] = 1.0
        carry_all[32 + c, c * NCHUNK:(c + 1) * NCHUNK] = 1.0
    carry_all = carry_all.astype(bf16)
    # DoubleRow one-hot stationaries for csum pairs: per pair u the two
    # halves select rows 2u and 2u+1 of the count psum
    onehot_dr = np.zeros((NCHUNK, NCH // 2, 2, NCH), np.float32)
    for u in range(NCH // 2):
        onehot_dr[:, u, 0, 2 * u] = 1.0
        onehot_dr[:, u, 1, 2 * u + 1] = 1.0
    onehot_dr = onehot_dr.reshape(NCHUNK, (NCH // 2) * 2 * NCH).astype(f8)
    iota_j = np.broadcast_to(np.arange(N, dtype=np.int16)[None, :], (128, N)).copy()
    ident = np.eye(128, dtype=np.float32)
    return tri_dr, carry_all, onehot_dr, iota_j, ident


def _kernel_body(tc, io):
    import concourse.bass as bass
    import concourse.mybir as mybir
    from contextlib import ExitStack

    nc = tc.nc
    dt = mybir.dt
    AF = mybir.ActivationFunctionType
    ALU = mybir.AluOpType

    with ExitStack() as ctx:
        const = ctx.enter_context(tc.tile_pool(name="const", bufs=1))
        w1t_sb = const.tile([128, 128], dt.bfloat16, tag="w1t")
        nc.sync.dma_start(out=w1t_sb, in_=io["w1t"])
        gamma_sb = const.tile([128, 1], dt.float32, tag="gm")
        nc.sync.dma_start(out=gamma_sb, in_=io["gamma"])
        beta_sb = const.tile([128, 1], dt.float32, tag="bt")
        nc.sync.dma_start(out=beta_sb, in_=io["beta"])
        pA_sb = const.tile([24, N], dt.bfloat16, tag="pA")
        nc.sync.dma_start(out=pA_sb, in_=io["pA"])
        pB_sb = const.tile([24, QPC], dt.bfloat16, tag="pB")
        nc.sync.dma_start(out=pB_sb, in_=io["pB"])
        tridr_sb = const.tile([128, 4 * NCHUNK], dt.float8e4, tag="tridr")
        nc.sync.dma_start(out=tridr_sb, in_=io["tri_dr"])
        tridr_v = tridr_sb.rearrange("p (two n) -> p two n", two=2)
        carry_sb = const.tile([64, NCH * 128], dt.bfloat16, tag="car")
        nc.sync.dma_start(out=carry_sb, in_=io["carry_all"])
        onehotdr_sb = const.tile([128, NCH * NCH], dt.float8e4, tag="ohdr")
        nc.sync.dma_start(out=onehotdr_sb, in_=io["onehot_dr"])
        iota_sb = const.tile([128, N], dt.int16, tag="iota")
        nc.sync.dma_start(out=iota_sb, in_=io["iota_j"])
        ident_sb = const.tile([128, 128], dt.float32, tag="idn")
        nc.sync.dma_start(out=ident_sb, in_=io["ident"])
        identb_sb = const.tile([128, 128], dt.bfloat16, tag="idb")
        nc.sync.dma_start(out=identb_sb, in_=io["ident_bf"])
        eps_t = const.tile([128, 1], dt.float32, tag="eps")
        nc.vector.memset(eps_t, BN_EPS)
        # sigmoid(-1e30*d^2 + 1e28) -> exact {0,1} validity indicator
        sgsc_t = const.tile([128, 1], dt.float32, tag="sgs")
        nc.vector.memset(sgsc_t, -1e30)
        sgbi_t = const.tile([128, 1], dt.float32, tag="sgb")
        nc.vector.memset(sgbi_t, R2 * 1e30)

        hpool = ctx.enter_context(tc.tile_pool(name="hp", bufs=1))
        h_n = hpool.tile([128, N], dt.float32, tag="hn")
        hT = hpool.tile([128, N], dt.float16, tag="hT")
        dbg_after_h = (lambda: nc.sync.dma_start(out=io["dbg_h"], in_=h_n)) \
            if "dbg_h" in io else (lambda: None)

        # ---------------- Phase A: BN stats + h_n ----------------
        ps_sq = ctx.enter_context(tc.tile_pool(name="ps_sq", bufs=2, space="PSUM"))
        with ExitStack() as actx:
            bigf = actx.enter_context(tc.tile_pool(name="bigf", bufs=1))
            ps_h = actx.enter_context(tc.tile_pool(name="ps_h", bufs=2, space="PSUM"))
            stp = actx.enter_context(tc.tile_pool(name="stp", bufs=1))

            f_sb = bigf.tile([128, B * N], dt.bfloat16, tag="f")
            nc.sync.dma_start(out=f_sb, in_=io["f_all"])
            b_idx = io["b_idx"]          # which batch this core owns

            stats = stp.tile([128, B * N // 512, 6], dt.float32, tag="st")
            for i in range(B * N // 512):
                ph = ps_h.tile([128, 512], dt.float32, tag="ph")
                nc.tensor.matmul(ph, w1t_sb, f_sb[:, i * 512:(i + 1) * 512],
                                 start=True, stop=True)
                nc.vector.bn_stats(out=stats[:, i, :], in_=ph)
            mv = stp.tile([128, 2], dt.float32, tag="mv")
            nc.vector.bn_aggr(out=mv, in_=stats)
            # rstd = 1/sqrt(var+eps); gamma2 = gamma*rstd; bias2 = beta - gamma2*mean
            sqv = stp.tile([128, 1], dt.float32, tag="sq")
            nc.scalar.activation(sqv, mv[:, 1:2], AF.Sqrt, bias=eps_t, scale=1.0)
            rstd = stp.tile([128, 1], dt.float32, tag="rs")
            nc.vector.reciprocal(rstd, sqv)
            gamma2 = stp.tile([128, 1], dt.float32, tag="g2")
            nc.vector.tensor_mul(gamma2, gamma_sb, rstd)
            gm = stp.tile([128, 1], dt.float32, tag="gmn")
            nc.vector.tensor_mul(gm, gamma2, mv[:, 0:1])
            bias2 = stp.tile([128, 1], dt.float32, tag="b2")
            nc.vector.tensor_sub(bias2, beta_sb, gm)

            for i in range(N // 512):
                ph = ps_h.tile([128, 512], dt.float32, tag="ph")
                nc.tensor.matmul(ph, w1t_sb,
                                 f_sb[:, b_idx * N + i * 512:b_idx * N + (i + 1) * 512],
                                 start=True, stop=True)
                nc.scalar.activation(h_n[:, i * 512:(i + 1) * 512], ph, AF.Relu,
                                     bias=bias2, scale=gamma2)

        dbg_after_h()

        # hT swizzled fp16 transpose in SBUF: hT[p, s*128+c] = h[c, s*128+p],
        # i.e. token n = (partition n%128, 256B stripe n//128) for dma_gather.
        with ExitStack() as tctx:
            ps_t = tctx.enter_context(tc.tile_pool(name="ps_t", bufs=2, space="PSUM"))
            for c2 in range(N // 128):
                tp2 = ps_t.tile([128, 128], dt.float32, tag="htp")
                nc.tensor.transpose(tp2, h_n[:, c2 * 128:(c2 + 1) * 128], ident_sb)
                nc.scalar.activation(hT[:, c2 * 128:(c2 + 1) * 128], tp2, AF.Copy)

        # ---------------- Phase B + C: half-split pipeline ----------------
        spool = ctx.enter_context(tc.tile_pool(name="sp", bufs=2))
        ps_cs = ctx.enter_context(tc.tile_pool(name="ps_cs", bufs=1, space="PSUM"))
        ps_tr = ctx.enter_context(tc.tile_pool(name="ps_tr", bufs=1, space="PSUM"))
        ps_rk = ctx.enter_context(tc.tile_pool(name="ps_rk", bufs=2, space="PSUM"))
        small = ctx.enter_context(tc.tile_pool(name="small", bufs=3))
        ctt_p = ctx.enter_context(tc.tile_pool(name="cttp", bufs=2))
        ipool = ctx.enter_context(tc.tile_pool(name="ip", bufs=8))
        dstp = ctx.enter_context(tc.tile_pool(name="dstp", bufs=2))
        iwpool = ctx.enter_context(tc.tile_pool(name="iw", bufs=1))
        fpool = ctx.enter_context(tc.tile_pool(name="fp", bufs=2))
        dpool = ctx.enter_context(tc.tile_pool(name="dp", bufs=2))
        opool = ctx.enter_context(tc.tile_pool(name="op", bufs=2))

        dlp_v = io["dlp_s"]          # [128, QPC*K] bf16, k reversed on host
        out_v = io["out_o"]          # [128, QPC] fp16

        zmask = small.tile([128, 256], dt.int8, tag="zm")
        nc.vector.memset(zmask, 0)

        def b_front(qb):
            """distances + per-chunk counts for one 512-query block."""
            qbs = slice(qb * QB, (qb + 1) * QB)
            s_t = []
            for c in range(NCH):
                psq = ps_sq.tile([128, QB], dt.float32, tag="sq")
                nc.tensor.matmul(psq, pA_sb[:, c * NCHUNK:(c + 1) * NCHUNK],
                                 pB_sb[:, qbs], start=True, stop=True)
                if c % 2 == 0:
                    sp2 = spool.tile([128, 2, QB], dt.float8e4, tag=f"s{c // 2}")
                    s_t.append(sp2)
                st = s_t[c // 2][:, c % 2, :]
                if c % 2 == 0:
                    nc.scalar.activation(st, psq, AF.Sigmoid, bias=sgbi_t,
                                         scale=sgsc_t)
                else:
                    nc.vector.tensor_scalar(st, psq, R2, None, op0=ALU.is_le)

            psC = ps_cs.tile([NCH, QB], dt.float32, tag="cs")
            for u in range(NCH // 2):
                nc.tensor.matmul(psC,
                                 onehotdr_sb[:, u * 2 * NCH:(u + 1) * 2 * NCH]
                                 .rearrange("p (two t) -> p two t", two=2),
                                 s_t[u], start=(u == 0), stop=(u == NCH // 2 - 1),
                                 perf_mode=mybir.MatmulPerfMode.DoubleRow)
            csT = small.tile([NCH, QB], dt.bfloat16, tag="csT")
            nc.scalar.activation(csT, psC, AF.Copy)
            csbs = []
            for qt in range(NQB):
                psc2 = ps_tr.tile([128, NCH], dt.bfloat16, tag="trp")
                nc.tensor.transpose(psc2, csT[:, qt * QT:(qt + 1) * QT],
                                    identb_sb[0:NCH, 0:NCH])
                csb = small.tile([128, NCH], dt.float32, tag=f"csb{qt}")
                nc.vector.tensor_copy(csb, psc2)
                csbs.append(csb)
            return s_t, csbs

        def b_rank(qb, qt, s_t, csbs):
            """slot ids for one 128-query tile -> idxs_sc [q, N] int16."""
            ti = qb * NQB + qt
            qs = slice(qt * QT, (qt + 1) * QT)
            csb = csbs[qt]
            pref = small.tile([128, NCH], dt.float32, tag="pf0")
            nc.vector.memset(pref[:, 0:1], 0.0)
            nc.vector.tensor_copy(pref[:, 1:], csb[:, :NCH - 1])
            for sh in (1, 2, 4, 8, 16):
                pref2 = small.tile([128, NCH], dt.float32, tag=f"pf{sh}")
                nc.vector.tensor_copy(pref2[:, :sh], pref[:, :sh])
                nc.vector.tensor_add(pref2[:, sh:], pref[:, sh:], pref[:, :NCH - sh])
                pref = pref2
            ct3 = small.tile([128, NCH], dt.float32, tag="ct3")
            nc.vector.tensor_scalar(ct3, pref, -1.0, -1.0,
                                    op0=ALU.mult, op1=ALU.add)
            ctpack = small.tile([128, 2 * NCH], dt.bfloat16, tag="ctp")
            nc.vector.tensor_copy(ctpack[:, :NCH], ct3)
            nc.vector.tensor_sub(ctpack[:, NCH:], ct3, ctpack[:, :NCH])
            psP = ps_tr.tile([2 * NCH, 128], dt.bfloat16, tag="ctpT")
            nc.tensor.transpose(psP, ctpack, identb_sb)
            ctt64 = ctt_p.tile([2 * NCH, 128], dt.bfloat16, tag="ctt")
            nc.scalar.activation(ctt64, psP, AF.Copy)

            idxs_sc = ipool.tile([128, N], dt.int16, tag="isc")
            for g in range(NGRP):
                pr = ps_rk.tile([128, GRP * 128], dt.float32, tag="rk")
                nc.tensor.matmul(pr, ctt64,
                                 carry_sb[:, g * GRP * 128:(g + 1) * GRP * 128],
                                 start=True, stop=False)
                for up in range(GRP // 2):
                    u = g * (GRP // 2) + up
                    nc.tensor.matmul(pr[:, up * 256:(up + 1) * 256],
                                     s_t[u][:, :, qs], tridr_v,
                                     start=False, stop=(up == GRP // 2 - 1),
                                     skip_group_check=(up != GRP // 2 - 1),
                                     perf_mode=mybir.MatmulPerfMode.DoubleRow)
                nc.scalar.activation(idxs_sc[:, g * GRP * 128:(g + 1) * GRP * 128],
                                     pr, AF.Copy)
            if "dbg_slots" in io and ti == 0:
                nc.sync.dma_start(out=io["dbg_slots"], in_=idxs_sc)
            return idxs_sc

        def b_scatter(ti, idxs_sc):
            """first-K extraction + wrapped idx layout for one tile."""
            dst = dstp.tile([128, K], dt.int16, tag="dst")
            nc.gpsimd.local_scatter(dst, iota_sb, idxs_sc,
                                    channels=128, num_elems=K, num_idxs=N)
            mask = small.tile([128, K], dt.int8, tag="msk")
            nc.vector.tensor_scalar(mask, dst, 0.0, None, op0=ALU.is_equal)
            nc.vector.copy_predicated(dst, mask, dst[:, K - 1:K].to_broadcast((128, K)))
            if "dbg_dst" in io:
                nc.sync.dma_start(out=io["dbg_dst"][:, ti * K:(ti + 1) * K], in_=dst)
            dstf = small.tile([128, 256], dt.float32, tag="dstf")
            dfv = dstf.rearrange("p (h r s) -> p h r s", h=2, r=8)
            dst_b = bass.AP(tensor=dst.tensor, offset=dst.offset,
                            ap=[dst.ap[0], [16, 2], [0, 8], [1, 16]])
            nc.vector.tensor_copy(dfv, dst_b)
            idxw = iwpool.tile([128, 256], dt.int16, tag=f"idxw{ti}")
            iwv = idxw.rearrange("p (c two) -> p c two", two=2)
            for half in range(2):
                tps = ps_tr.tile([128, 128], dt.float32, tag="tp")
                nc.tensor.transpose(tps, dstf[:, half * 128:(half + 1) * 128],
                                    ident_sb)
                nc.scalar.activation(iwv[:, :, half], tps, AF.Copy)
            return idxw, dst

        def c_order(tiles):
            """force this half's gathers after its last local_scatter."""
            last_dst = tiles[-1][1]
            for idxw, _ in tiles:
                nc.vector.copy_predicated(idxw, zmask,
                                          last_dst[:, 0:1].to_broadcast((128, 256)))

        def c_tile(ti, idxw):
            """gather + add + max-pool for one tile."""
            fj = fpool.tile([128, QT * K], dt.float16, tag="fj")
            fj_v = fj.rearrange("p (o i) -> p o i", o=1)
            for gc in range(QT * K // GC):
                nc.gpsimd.dma_gather(
                    fj_v[:, :, gc * GC:(gc + 1) * GC], hT,
                    idxw[:, gc * (GC // 16):(gc + 1) * (GC // 16)],
                    num_idxs=GC, num_idxs_reg=GC, elem_size=128,
                    transpose=True,
                    sbuf_tokens_per_rank=128,
                    sbuf_free_dim_per_rank=256,
                    queue_num=(ti * (QT * K // GC) + gc) % 2)
            if "dbg_fj" in io and ti == 0:
                nc.sync.dma_start(out=io["dbg_fj"], in_=fj)
            dlp_t = dpool.tile([128, QT * K], dt.bfloat16, tag="dl")
            nc.sync.dma_start(out=dlp_t,
                              in_=dlp_v[:, ti * QT * K:(ti + 1) * QT * K])
            nc.vector.tensor_add(fj, fj, dlp_t)
            out_t = opool.tile([128, QT], dt.float16, tag="ot")
            nc.vector.tensor_reduce(out_t,
                                    fj.rearrange("p (q k) -> p q k", k=K),
                                    axis=mybir.AxisListType.X, op=ALU.max)
            nc.sync.dma_start(out=out_v[:, ti * QT:(ti + 1) * QT], in_=out_t)

        # ---- half A (tiles 0-7): ball query + scatters (gpsimd lib 7) ----
        tiles_a = []
        for qb in (0, 1):
            s_t, csbs = b_front(qb)
            for qt in range(NQB):
                ti = qb * NQB + qt
                tiles_a.append(b_scatter(ti, b_rank(qb, qt, s_t, csbs)))
        c_order(tiles_a)

        # ---- half A gathers (lib 3) interleaved with half-B rank compute ----
        # (interleaved EMISSION so the per-engine program order doesn't stall
        # half-B's PE/DVE/Act work behind the slow half-A gathers)
        ranks_b = []
        for qb in (2, 3):
            s_t, csbs = b_front(qb)
            for qt in range(NQB):
                ti_a = tiles_a[(qb - 2) * NQB + qt][0]
                c_tile((qb - 2) * NQB + qt, ti_a)
                ranks_b.append(b_rank(qb, qt, s_t, csbs))

        # fence: half-B local_scatters (lib 7) must wait for all half-A
        # gathers (lib 3). Touch every half-A idxw after its gathers (WAR),
        # then thread those touches into iota (RAW) so the scatters (iota
        # readers) are ordered after.
        for i, (idxw, _) in enumerate(tiles_a):
            nc.vector.copy_predicated(idxw, zmask,
                                      tiles_a[-1][1][:, 0:1].to_broadcast((128, 256)))
            nc.vector.copy_predicated(iota_sb[:, i * 256:(i + 1) * 256],
                                      zmask, idxw)

        # ---- half B scatters (lib 7), then gathers (lib 3) ----
        tiles_b = [b_scatter(8 + i, r) for i, r in enumerate(ranks_b)]
        c_order(tiles_b)
        for i, (idxw, _) in enumerate(tiles_b):
            c_tile(8 + i, idxw)


@functools.lru_cache(maxsize=1)
def _compiled():
    import concourse.bacc as bacc
    import concourse.tile as tile
    import concourse.mybir as mybir

    dt = mybir.dt
    nc = bacc.Bacc("TRN2", target_bir_lowering=False, debug=False,
                   num_devices=NCORES, num_swdge_queues=2)
    io = {}

    def din(name, shape, dtype):
        io[name] = nc.dram_tensor(name, shape, dtype, kind="ExternalInput").ap()

    din("f_all", [128, B * N], dt.bfloat16)
    din("w1t", [128, 128], dt.bfloat16)
    din("gamma", [128, 1], dt.float32)
    din("beta", [128, 1], dt.float32)
    din("pA", [24, N], dt.bfloat16)
    din("pB", [24, QPC], dt.bfloat16)
    din("tri_dr", [128, 4 * NCHUNK], dt.float8e4)
    din("carry_all", [64, NCH * 128], dt.bfloat16)
    din("onehot_dr", [128, NCH * NCH], dt.float8e4)
    din("iota_j", [128, N], dt.int16)
    din("ident", [128, 128], dt.float32)
    din("ident_bf", [128, 128], dt.bfloat16)
    din("dlp_s", [128, QPC * K], dt.bfloat16)
    io["out_o"] = nc.dram_tensor("out_o", [128, QPC], dt.float16,
                                 kind="ExternalOutput").ap()
    import os
    if os.environ.get("KM_DEBUG"):
        io["dbg_h"] = nc.dram_tensor("dbg_h", [128, N], dt.float32,
                                     kind="ExternalOutput").ap()
        io["dbg_slots"] = nc.dram_tensor("dbg_slots", [128, N], dt.int16,
                                         kind="ExternalOutput").ap()
        io["dbg_dst"] = nc.dram_tensor("dbg_dst", [128, NQT * K], dt.int16,
                                       kind="ExternalOutput").ap()
        io["dbg_fj"] = nc.dram_tensor("dbg_fj", [128, QT * K], dt.float16,
                                      kind="ExternalOutput").ap()

    # All cores run the same program; the host rotates f_all so each core's
    # own batch occupies columns [0, N), hence b_idx is constant 0.
    io["b_idx"] = 0

    with tile.TileContext(nc) as tc:
        _kernel_body(tc, io)
    nc.compile()
    return nc


def _host_prep(inputs):
    p = np.asarray(inputs["p"], np.float32)
    f = np.asarray(inputs["f"], np.float32)
    dlp = np.asarray(inputs["dlp"], np.float32)
    W1 = np.asarray(inputs["W1"], np.float32)
    gamma = np.asarray(inputs["gamma"], np.float32)
    beta = np.asarray(inputs["beta"], np.float32)

    tri_dr, carry_all, onehot_dr, iota_j, ident = _static_tables()
    f_all = np.ascontiguousarray(np.moveaxis(f, 0, 1).reshape(C, B * N))
    w1t = np.ascontiguousarray(W1.T).astype(bf16)

    in_maps = []
    for core in range(NCORES):
        b, half = core // 2, core % 2
        q_lo, q_hi = half * QPC, (half + 1) * QPC
        pA, pB = _build_AB(p[b], q_lo, q_hi)
        dlp_s = np.ascontiguousarray(
            dlp[b, :, q_lo:q_hi, ::-1].reshape(C, QPC * K)).astype(bf16)
        # rotate f so this core's batch occupies columns [0, N)
        f_rot = np.ascontiguousarray(
            np.concatenate([f_all[:, b * N:], f_all[:, :b * N]], axis=1)
        ).astype(bf16)
        in_maps.append({
            "f_all": f_rot,
            "w1t": w1t,
            "gamma": np.ascontiguousarray(gamma.reshape(C, 1)),
            "beta": np.ascontiguousarray(beta.reshape(C, 1)),
            "pA": pA, "pB": pB,
            "tri_dr": tri_dr, "carry_all": carry_all,
            "onehot_dr": onehot_dr, "iota_j": iota_j, "ident": ident,
            "ident_bf": ident.astype(bf16),
            "dlp_s": dlp_s,
        })
    return in_maps


def run(inputs, trace=False, **kw):
    from concourse.bass_utils import run_bass_kernel_spmd
    nc = _compiled()
    in_maps = _host_prep(inputs)
    res = run_bass_kernel_spmd(nc, in_maps, core_ids=list(range(NCORES)),
                               trace=trace, **kw)
    out = np.zeros((B, C, N), np.float32)
    for core in range(NCORES):
        b, half = core // 2, core % 2
        out[b, :, half * QPC:(half + 1) * QPC] = \
            res.results[core]["out_o"].astype(np.float32)
    return out, res


def kernel(**inputs) -> np.ndarray:
    out, _ = run(inputs, trace=False)
    return out


# revision 26
# speedup vs baseline: 1.1578x; 1.1578x over previous
"""Trainium2 Bass kernel for nn_DGASEncoder (PointNet++-style ball-query encoder).

Self-contained: hardcoded shapes; takes FULL inputs, shards across 8 NeuronCores
(data-parallel over (batch, N/2)), returns the FULL output.

Per-core pipeline (SPMD; cores differ only in input data):
  A) conv1d+BN stats: h = W1 @ f (bf16) over all B*N via PE; bn_stats/bn_aggr ->
     mean/var; h_n = relu(gamma'*h + beta') f32; swizzled fp16 transpose hT in
     SBUF (token n = partition n%128, stripe n//128).
  B) ball query per 128-query tile: squared distances via 24-row bf16-split
     matmuls (f32-exact); indicator on ACT/DVE; per-chunk valid counts via
     one-hot-column stationary matmuls; ranks via per-chunk triangular matmuls
     + carry matmuls; slot ids evacuated as int16; first-K extraction with
     gpsimd local_scatter (lib 7).
  C) neighbor features via SWDGE dma_gather (lib 3) straight out of SBUF hT
     (no DRAM round-trip); fused add with bf16 dlp stream + max-pool over K on
     DVE (fp16, 2x mode). All scatters are forced before the first gather so
     the gpsimd IRAM library is switched exactly once.
"""
import functools
import numpy as np
import ml_dtypes

B, N, C, K = 4, 4096, 128, 32
RADIUS = 0.1
BN_EPS = 1e-5
R2 = RADIUS * RADIUS
NCHUNK = 128
NCH = N // NCHUNK          # 32
QPC = N // 2               # 2048 queries per core
QT = 128                   # queries per tile
NQT = QPC // QT            # 16
QB = 512                   # query block
NQB = QPC // QB            # 4
GRP = 4                    # chunks per rank-psum group
NGRP = NCH // GRP          # 8
NCORES = 8
GC = 512                   # gather chunk (idxs per dma_gather)

bf16 = ml_dtypes.bfloat16


def _bf(x):
    return np.asarray(x, dtype=bf16).astype(np.float32)


def _split3(x):
    h = _bf(x)
    m = _bf(x - h)
    l = _bf(x - h - m)
    return h, m, l


def _build_AB(p_b, q_lo, q_hi):
    """A [24, N] (candidate side, -2 scale folded), Bm [24, Q] (query side), bf16."""
    x = p_b.astype(np.float32)
    pn = np.sum(x * x, axis=1, dtype=np.float32)
    sp = [_split3(x[:, d]) for d in range(3)]
    sp2 = [tuple(-2.0 * t for t in sp[d]) for d in range(3)]
    spn = _split3(pn)
    PAIRS = [(0, 0), (0, 1), (1, 0), (0, 2), (2, 0), (1, 1)]
    A_rows, B_rows = [], []
    for d in range(3):
        for (s, s2) in PAIRS:
            A_rows.append(sp2[d][s])
            B_rows.append(sp[d][s2][q_lo:q_hi])
    for s in range(3):
        A_rows.append(spn[s])
        B_rows.append(np.ones(q_hi - q_lo, np.float32))
    for s in range(3):
        A_rows.append(np.ones(N, np.float32))
        B_rows.append(spn[s][q_lo:q_hi])
    return (np.stack(A_rows).astype(bf16), np.stack(B_rows).astype(bf16))


def _static_tables():
    f8 = ml_dtypes.float8_e4m3fn
    # tri weights for v in {0,1}: slot contribution 32*v_t - (#valid before t)
    tri = np.zeros((NCHUNK, NCHUNK), np.float32)
    for jl in range(NCHUNK):
        tri[:jl, jl] = -1.0
        tri[jl, jl] = 32.0
    # DoubleRow moving for rank pairs: X0 = [tri | 0], X1 = [0 | tri]
    # packed two-major as [128, 512]
    tri_dr = np.zeros((NCHUNK, 2, 2 * NCHUNK), np.float32)
    tri_dr[:, 0, :NCHUNK] = tri
    tri_dr[:, 1, NCHUNK:] = tri
    tri_dr = tri_dr.reshape(NCHUNK, 4 * NCHUNK).astype(f8)
    # carry table [64, NCH*128]: rows 0..31 block-diag ones (ct3 hi), rows
    # 32..63 block-diag ones (ct3 lo); ct3 = -CNT_<c - 1.
    carry_all = np.zeros((64, NCH * NCHUNK), np.float32)
    for c in range(NCH):
        carry_all[c, c * NCHUNK:(c + 1) * NCHUNK] = 1.0
        carry_all[32 + c, c * NCHUNK:(c + 1) * NCHUNK] = 1.0
    carry_all = carry_all.astype(bf16)
    # DoubleRow one-hot stationaries for csum pairs: per pair u the two
    # halves select rows 2u and 2u+1 of the count psum
    onehot_dr = np.zeros((NCHUNK, NCH // 2, 2, NCH), np.float32)
    for u in range(NCH // 2):
        onehot_dr[:, u, 0, 2 * u] = 1.0
        onehot_dr[:, u, 1, 2 * u + 1] = 1.0
    onehot_dr = onehot_dr.reshape(NCHUNK, (NCH // 2) * 2 * NCH).astype(f8)
    iota_j = np.broadcast_to(np.arange(N, dtype=np.int16)[None, :], (128, N)).copy()
    ident = np.eye(128, dtype=np.float32)
    return tri_dr, carry_all, onehot_dr, iota_j, ident


def _kernel_body(tc, io):
    import concourse.bass as bass
    import concourse.mybir as mybir
    from contextlib import ExitStack

    nc = tc.nc
    dt = mybir.dt
    AF = mybir.ActivationFunctionType
    ALU = mybir.AluOpType

    with ExitStack() as ctx:
        const = ctx.enter_context(tc.tile_pool(name="const", bufs=1))
        w1t_sb = const.tile([128, 128], dt.bfloat16, tag="w1t")
        nc.sync.dma_start(out=w1t_sb, in_=io["w1t"])
        gamma_sb = const.tile([128, 1], dt.float32, tag="gm")
        nc.sync.dma_start(out=gamma_sb, in_=io["gamma"])
        beta_sb = const.tile([128, 1], dt.float32, tag="bt")
        nc.sync.dma_start(out=beta_sb, in_=io["beta"])
        pA_sb = const.tile([24, N], dt.bfloat16, tag="pA")
        nc.sync.dma_start(out=pA_sb, in_=io["pA"])
        pB_sb = const.tile([24, QPC], dt.bfloat16, tag="pB")
        nc.sync.dma_start(out=pB_sb, in_=io["pB"])
        tridr_sb = const.tile([128, 4 * NCHUNK], dt.float8e4, tag="tridr")
        nc.sync.dma_start(out=tridr_sb, in_=io["tri_dr"])
        tridr_v = tridr_sb.rearrange("p (two n) -> p two n", two=2)
        carry_sb = const.tile([64, NCH * 128], dt.bfloat16, tag="car")
        nc.sync.dma_start(out=carry_sb, in_=io["carry_all"])
        onehotdr_sb = const.tile([128, NCH * NCH], dt.float8e4, tag="ohdr")
        nc.sync.dma_start(out=onehotdr_sb, in_=io["onehot_dr"])
        iota_sb = const.tile([128, N], dt.int16, tag="iota")
        nc.sync.dma_start(out=iota_sb, in_=io["iota_j"])
        ident_sb = const.tile([128, 128], dt.float32, tag="idn")
        nc.sync.dma_start(out=ident_sb, in_=io["ident"])
        identb_sb = const.tile([128, 128], dt.bfloat16, tag="idb")
        nc.sync.dma_start(out=identb_sb, in_=io["ident_bf"])
        eps_t = const.tile([128, 1], dt.float32, tag="eps")
        nc.vector.memset(eps_t, BN_EPS)
        # sigmoid(-1e30*d^2 + 1e28) -> exact {0,1} validity indicator
        sgsc_t = const.tile([128, 1], dt.float32, tag="sgs")
        nc.vector.memset(sgsc_t, -1e30)
        sgbi_t = const.tile([128, 1], dt.float32, tag="sgb")
        nc.vector.memset(sgbi_t, R2 * 1e30)

        hpool = ctx.enter_context(tc.tile_pool(name="hp", bufs=1))
        h_n = hpool.tile([128, N], dt.float32, tag="hn")
        hT = hpool.tile([128, N], dt.float16, tag="hT")
        dbg_after_h = (lambda: nc.sync.dma_start(out=io["dbg_h"], in_=h_n)) \
            if "dbg_h" in io else (lambda: None)

        # ---------------- Phase A: BN stats + h_n ----------------
        ps_sq = ctx.enter_context(tc.tile_pool(name="ps_sq", bufs=2, space="PSUM"))
        with ExitStack() as actx:
            bigf = actx.enter_context(tc.tile_pool(name="bigf", bufs=1))
            ps_h = actx.enter_context(tc.tile_pool(name="ps_h", bufs=2, space="PSUM"))
            stp = actx.enter_context(tc.tile_pool(name="stp", bufs=1))

            f_sb = bigf.tile([128, B * N], dt.bfloat16, tag="f")
            nc.sync.dma_start(out=f_sb, in_=io["f_all"])
            b_idx = io["b_idx"]          # which batch this core owns

            stats = stp.tile([128, B * N // 512, 6], dt.float32, tag="st")
            for i in range(B * N // 512):
                ph = ps_h.tile([128, 512], dt.float32, tag="ph")
                nc.tensor.matmul(ph, w1t_sb, f_sb[:, i * 512:(i + 1) * 512],
                                 start=True, stop=True)
                nc.vector.bn_stats(out=stats[:, i, :], in_=ph)
            mv = stp.tile([128, 2], dt.float32, tag="mv")
            nc.vector.bn_aggr(out=mv, in_=stats)
            # rstd = 1/sqrt(var+eps); gamma2 = gamma*rstd; bias2 = beta - gamma2*mean
            sqv = stp.tile([128, 1], dt.float32, tag="sq")
            nc.scalar.activation(sqv, mv[:, 1:2], AF.Sqrt, bias=eps_t, scale=1.0)
            rstd = stp.tile([128, 1], dt.float32, tag="rs")
            nc.vector.reciprocal(rstd, sqv)
            gamma2 = stp.tile([128, 1], dt.float32, tag="g2")
            nc.vector.tensor_mul(gamma2, gamma_sb, rstd)
            gm = stp.tile([128, 1], dt.float32, tag="gmn")
            nc.vector.tensor_mul(gm, gamma2, mv[:, 0:1])
            bias2 = stp.tile([128, 1], dt.float32, tag="b2")
            nc.vector.tensor_sub(bias2, beta_sb, gm)

            for i in range(N // 512):
                ph = ps_h.tile([128, 512], dt.float32, tag="ph")
                nc.tensor.matmul(ph, w1t_sb,
                                 f_sb[:, b_idx * N + i * 512:b_idx * N + (i + 1) * 512],
                                 start=True, stop=True)
                nc.scalar.activation(h_n[:, i * 512:(i + 1) * 512], ph, AF.Relu,
                                     bias=bias2, scale=gamma2)

        dbg_after_h()

        # hT swizzled fp16 transpose in SBUF: hT[p, s*128+c] = h[c, s*128+p],
        # i.e. token n = (partition n%128, 256B stripe n//128) for dma_gather.
        with ExitStack() as tctx:
            ps_t = tctx.enter_context(tc.tile_pool(name="ps_t", bufs=2, space="PSUM"))
            for c2 in range(N // 128):
                tp2 = ps_t.tile([128, 128], dt.float32, tag="htp")
                nc.tensor.transpose(tp2, h_n[:, c2 * 128:(c2 + 1) * 128], ident_sb)
                nc.scalar.activation(hT[:, c2 * 128:(c2 + 1) * 128], tp2, AF.Copy)

        # ---------------- Phase B + C: half-split pipeline ----------------
        spool = ctx.enter_context(tc.tile_pool(name="sp", bufs=2))
        ps_cs = ctx.enter_context(tc.tile_pool(name="ps_cs", bufs=1, space="PSUM"))
        ps_tr = ctx.enter_context(tc.tile_pool(name="ps_tr", bufs=1, space="PSUM"))
        ps_rk = ctx.enter_context(tc.tile_pool(name="ps_rk", bufs=2, space="PSUM"))
        small = ctx.enter_context(tc.tile_pool(name="small", bufs=3))
        ctt_p = ctx.enter_context(tc.tile_pool(name="cttp", bufs=2))
        ipool = ctx.enter_context(tc.tile_pool(name="ip", bufs=2))
        dstp = ctx.enter_context(tc.tile_pool(name="dstp", bufs=2))
        iwpool = ctx.enter_context(tc.tile_pool(name="iw", bufs=1))
        fpool = ctx.enter_context(tc.tile_pool(name="fp", bufs=2))
        dpool = ctx.enter_context(tc.tile_pool(name="dp", bufs=2))
        opool = ctx.enter_context(tc.tile_pool(name="op", bufs=2))

        dlp_v = io["dlp_s"]          # [128, QPC*K] bf16, k reversed on host
        out_v = io["out_o"]          # [128, QPC] fp16

        zmask = small.tile([128, 256], dt.int8, tag="zm")
        nc.vector.memset(zmask, 0)

        def b_front(qb):
            """distances + per-chunk counts for one 512-query block."""
            qbs = slice(qb * QB, (qb + 1) * QB)
            s_t = []
            for c in range(NCH):
                psq = ps_sq.tile([128, QB], dt.float32, tag="sq")
                nc.tensor.matmul(psq, pA_sb[:, c * NCHUNK:(c + 1) * NCHUNK],
                                 pB_sb[:, qbs], start=True, stop=True)
                if c % 2 == 0:
                    sp2 = spool.tile([128, 2, QB], dt.float8e4, tag=f"s{c // 2}")
                    s_t.append(sp2)
                st = s_t[c // 2][:, c % 2, :]
                if c % 2 == 0:
                    nc.scalar.activation(st, psq, AF.Sigmoid, bias=sgbi_t,
                                         scale=sgsc_t)
                else:
                    nc.vector.tensor_scalar(st, psq, R2, None, op0=ALU.is_le)

            psC = ps_cs.tile([NCH, QB], dt.float32, tag="cs")
            for u in range(NCH // 2):
                nc.tensor.matmul(psC,
                                 onehotdr_sb[:, u * 2 * NCH:(u + 1) * 2 * NCH]
                                 .rearrange("p (two t) -> p two t", two=2),
                                 s_t[u], start=(u == 0), stop=(u == NCH // 2 - 1),
                                 perf_mode=mybir.MatmulPerfMode.DoubleRow)
            csT = small.tile([NCH, QB], dt.bfloat16, tag="csT")
            nc.scalar.activation(csT, psC, AF.Copy)
            csbs = []
            for qt in range(NQB):
                psc2 = ps_tr.tile([128, NCH], dt.bfloat16, tag="trp")
                nc.tensor.transpose(psc2, csT[:, qt * QT:(qt + 1) * QT],
                                    identb_sb[0:NCH, 0:NCH])
                csb = small.tile([128, NCH], dt.float32, tag=f"csb{qt}")
                nc.vector.tensor_copy(csb, psc2)
                csbs.append(csb)
            return s_t, csbs

        def b_rank(qb, qt, s_t, csbs):
            """slot ids for one 128-query tile -> idxs_sc [q, N] int16."""
            ti = qb * NQB + qt
            qs = slice(qt * QT, (qt + 1) * QT)
            csb = csbs[qt]
            pref = small.tile([128, NCH], dt.float32, tag="pf0")
            nc.vector.memset(pref[:, 0:1], 0.0)
            nc.vector.tensor_copy(pref[:, 1:], csb[:, :NCH - 1])
            for sh in (1, 2, 4, 8, 16):
                pref2 = small.tile([128, NCH], dt.float32, tag=f"pf{sh}")
                nc.vector.tensor_copy(pref2[:, :sh], pref[:, :sh])
                nc.vector.tensor_add(pref2[:, sh:], pref[:, sh:], pref[:, :NCH - sh])
                pref = pref2
            ct3 = small.tile([128, NCH], dt.float32, tag="ct3")
            nc.vector.tensor_scalar(ct3, pref, -1.0, -1.0,
                                    op0=ALU.mult, op1=ALU.add)
            ctpack = small.tile([128, 2 * NCH], dt.bfloat16, tag="ctp")
            nc.vector.tensor_copy(ctpack[:, :NCH], ct3)
            nc.vector.tensor_sub(ctpack[:, NCH:], ct3, ctpack[:, :NCH])
            psP = ps_tr.tile([2 * NCH, 128], dt.bfloat16, tag="ctpT")
            nc.tensor.transpose(psP, ctpack, identb_sb)
            ctt64 = ctt_p.tile([2 * NCH, 128], dt.bfloat16, tag="ctt")
            nc.scalar.activation(ctt64, psP, AF.Copy)

            idxs_sc = ipool.tile([128, N], dt.int16, tag="isc")
            for g in range(NGRP):
                pr = ps_rk.tile([128, GRP * 128], dt.float32, tag="rk")
                nc.tensor.matmul(pr, ctt64,
                                 carry_sb[:, g * GRP * 128:(g + 1) * GRP * 128],
                                 start=True, stop=False)
                for up in range(GRP // 2):
                    u = g * (GRP // 2) + up
                    nc.tensor.matmul(pr[:, up * 256:(up + 1) * 256],
                                     s_t[u][:, :, qs], tridr_v,
                                     start=False, stop=(up == GRP // 2 - 1),
                                     skip_group_check=(up != GRP // 2 - 1),
                                     perf_mode=mybir.MatmulPerfMode.DoubleRow)
                nc.scalar.activation(idxs_sc[:, g * GRP * 128:(g + 1) * GRP * 128],
                                     pr, AF.Copy)
            if "dbg_slots" in io and ti == 0:
                nc.sync.dma_start(out=io["dbg_slots"], in_=idxs_sc)
            return idxs_sc

        def b_scatter(ti, idxs_sc):
            """first-K extraction + wrapped idx layout for one tile."""
            dst = dstp.tile([128, K], dt.int16, tag="dst")
            nc.gpsimd.local_scatter(dst, iota_sb, idxs_sc,
                                    channels=128, num_elems=K, num_idxs=N)
            mask = small.tile([128, K], dt.int8, tag="msk")
            nc.vector.tensor_scalar(mask, dst, 0.0, None, op0=ALU.is_equal)
            nc.vector.copy_predicated(dst, mask, dst[:, K - 1:K].to_broadcast((128, K)))
            if "dbg_dst" in io:
                nc.sync.dma_start(out=io["dbg_dst"][:, ti * K:(ti + 1) * K], in_=dst)
            dstf = small.tile([128, 256], dt.float32, tag="dstf")
            dfv = dstf.rearrange("p (h r s) -> p h r s", h=2, r=8)
            dst_b = bass.AP(tensor=dst.tensor, offset=dst.offset,
                            ap=[dst.ap[0], [16, 2], [0, 8], [1, 16]])
            nc.vector.tensor_copy(dfv, dst_b)
            idxw = iwpool.tile([128, 256], dt.int16, tag=f"idxw{ti}")
            iwv = idxw.rearrange("p (c two) -> p c two", two=2)
            for half in range(2):
                tps = ps_tr.tile([128, 128], dt.float32, tag="tp")
                nc.tensor.transpose(tps, dstf[:, half * 128:(half + 1) * 128],
                                    ident_sb)
                nc.scalar.activation(iwv[:, :, half], tps, AF.Copy)
            return idxw, dst

        def c_order(tiles):
            """force this half's gathers after its last local_scatter."""
            last_dst = tiles[-1][1]
            for idxw, _ in tiles:
                nc.vector.copy_predicated(idxw, zmask,
                                          last_dst[:, 0:1].to_broadcast((128, 256)))

        def c_tile(ti, idxw):
            """gather + add + max-pool for one tile."""
            fj = fpool.tile([128, QT * K], dt.float16, tag="fj")
            fj_v = fj.rearrange("p (o i) -> p o i", o=1)
            for gc in range(QT * K // GC):
                nc.gpsimd.dma_gather(
                    fj_v[:, :, gc * GC:(gc + 1) * GC], hT,
                    idxw[:, gc * (GC // 16):(gc + 1) * (GC // 16)],
                    num_idxs=GC, num_idxs_reg=GC, elem_size=128,
                    transpose=True,
                    sbuf_tokens_per_rank=128,
                    sbuf_free_dim_per_rank=256,
                    queue_num=(ti * (QT * K // GC) + gc) % 2)
            if "dbg_fj" in io and ti == 0:
                nc.sync.dma_start(out=io["dbg_fj"], in_=fj)
            dlp_t = dpool.tile([128, QT * K], dt.bfloat16, tag="dl")
            nc.sync.dma_start(out=dlp_t,
                              in_=dlp_v[:, ti * QT * K:(ti + 1) * QT * K])
            nc.vector.tensor_add(fj, fj, dlp_t)
            out_t = opool.tile([128, QT], dt.float16, tag="ot")
            nc.vector.tensor_reduce(out_t,
                                    fj.rearrange("p (q k) -> p q k", k=K),
                                    axis=mybir.AxisListType.X, op=ALU.max)
            nc.sync.dma_start(out=out_v[:, ti * QT:(ti + 1) * QT], in_=out_t)

        # ---- phase B for all tiles (scatters on gpsimd lib 7) ----
        tiles = []
        for qb in range(NQB):
            s_t, csbs = b_front(qb)
            for qt in range(NQB):
                ti = qb * NQB + qt
                tiles.append(b_scatter(ti, b_rank(qb, qt, s_t, csbs)))

        # ---- phase C for all tiles (gathers on gpsimd lib 3) ----
        c_order(tiles)
        for ti, (idxw, _) in enumerate(tiles):
            c_tile(ti, idxw)


@functools.lru_cache(maxsize=1)
def _compiled():
    import concourse.bacc as bacc
    import concourse.tile as tile
    import concourse.mybir as mybir

    dt = mybir.dt
    nc = bacc.Bacc("TRN2", target_bir_lowering=False, debug=False,
                   num_devices=NCORES, num_swdge_queues=2)
    io = {}

    def din(name, shape, dtype):
        io[name] = nc.dram_tensor(name, shape, dtype, kind="ExternalInput").ap()

    din("f_all", [128, B * N], dt.bfloat16)
    din("w1t", [128, 128], dt.bfloat16)
    din("gamma", [128, 1], dt.float32)
    din("beta", [128, 1], dt.float32)
    din("pA", [24, N], dt.bfloat16)
    din("pB", [24, QPC], dt.bfloat16)
    din("tri_dr", [128, 4 * NCHUNK], dt.float8e4)
    din("carry_all", [64, NCH * 128], dt.bfloat16)
    din("onehot_dr", [128, NCH * NCH], dt.float8e4)
    din("iota_j", [128, N], dt.int16)
    din("ident", [128, 128], dt.float32)
    din("ident_bf", [128, 128], dt.bfloat16)
    din("dlp_s", [128, QPC * K], dt.bfloat16)
    io["out_o"] = nc.dram_tensor("out_o", [128, QPC], dt.float16,
                                 kind="ExternalOutput").ap()
    import os
    if os.environ.get("KM_DEBUG"):
        io["dbg_h"] = nc.dram_tensor("dbg_h", [128, N], dt.float32,
                                     kind="ExternalOutput").ap()
        io["dbg_slots"] = nc.dram_tensor("dbg_slots", [128, N], dt.int16,
                                         kind="ExternalOutput").ap()
        io["dbg_dst"] = nc.dram_tensor("dbg_dst", [128, NQT * K], dt.int16,
                                       kind="ExternalOutput").ap()
        io["dbg_fj"] = nc.dram_tensor("dbg_fj", [128, QT * K], dt.float16,
                                      kind="ExternalOutput").ap()

    # All cores run the same program; the host rotates f_all so each core's
    # own batch occupies columns [0, N), hence b_idx is constant 0.
    io["b_idx"] = 0

    with tile.TileContext(nc) as tc:
        _kernel_body(tc, io)
    nc.compile()
    return nc


def _host_prep(inputs):
    p = np.asarray(inputs["p"], np.float32)
    f = np.asarray(inputs["f"], np.float32)
    dlp = np.asarray(inputs["dlp"], np.float32)
    W1 = np.asarray(inputs["W1"], np.float32)
    gamma = np.asarray(inputs["gamma"], np.float32)
    beta = np.asarray(inputs["beta"], np.float32)

    tri_dr, carry_all, onehot_dr, iota_j, ident = _static_tables()
    f_all = np.ascontiguousarray(np.moveaxis(f, 0, 1).reshape(C, B * N))
    w1t = np.ascontiguousarray(W1.T).astype(bf16)

    in_maps = []
    for core in range(NCORES):
        b, half = core // 2, core % 2
        q_lo, q_hi = half * QPC, (half + 1) * QPC
        pA, pB = _build_AB(p[b], q_lo, q_hi)
        dlp_s = np.ascontiguousarray(
            dlp[b, :, q_lo:q_hi, ::-1].reshape(C, QPC * K)).astype(bf16)
        # rotate f so this core's batch occupies columns [0, N)
        f_rot = np.ascontiguousarray(
            np.concatenate([f_all[:, b * N:], f_all[:, :b * N]], axis=1)
        ).astype(bf16)
        in_maps.append({
            "f_all": f_rot,
            "w1t": w1t,
            "gamma": np.ascontiguousarray(gamma.reshape(C, 1)),
            "beta": np.ascontiguousarray(beta.reshape(C, 1)),
            "pA": pA, "pB": pB,
            "tri_dr": tri_dr, "carry_all": carry_all,
            "onehot_dr": onehot_dr, "iota_j": iota_j, "ident": ident,
            "ident_bf": ident.astype(bf16),
            "dlp_s": dlp_s,
        })
    return in_maps


def run(inputs, trace=False, **kw):
    from concourse.bass_utils import run_bass_kernel_spmd
    nc = _compiled()
    in_maps = _host_prep(inputs)
    res = run_bass_kernel_spmd(nc, in_maps, core_ids=list(range(NCORES)),
                               trace=trace, **kw)
    out = np.zeros((B, C, N), np.float32)
    for core in range(NCORES):
        b, half = core // 2, core % 2
        out[b, :, half * QPC:(half + 1) * QPC] = \
            res.results[core]["out_o"].astype(np.float32)
    return out, res


def kernel(**inputs) -> np.ndarray:
    out, _ = run(inputs, trace=False)
    return out


# revision 27
# speedup vs baseline: 1.1597x; 1.0016x over previous
"""Trainium2 Bass kernel for nn_DGASEncoder (PointNet++-style ball-query encoder).

Self-contained: hardcoded shapes; takes FULL inputs, shards across 8 NeuronCores
(data-parallel over (batch, N/2)), returns the FULL output.

Per-core pipeline (SPMD; cores differ only in input data):
  A) conv1d+BN stats: h = W1 @ f (bf16) over all B*N via PE; bn_stats/bn_aggr ->
     mean/var; h_n = relu(gamma'*h + beta') f32; swizzled fp16 transpose hT in
     SBUF (token n = partition n%128, stripe n//128).
  B) ball query per 128-query tile: squared distances via 24-row bf16-split
     matmuls (f32-exact); indicator on ACT/DVE; per-chunk valid counts via
     one-hot-column stationary matmuls; ranks via per-chunk triangular matmuls
     + carry matmuls; slot ids evacuated as int16; first-K extraction with
     gpsimd local_scatter (lib 7).
  C) neighbor features via SWDGE dma_gather (lib 3) straight out of SBUF hT
     (no DRAM round-trip); fused add with bf16 dlp stream + max-pool over K on
     DVE (fp16, 2x mode). All scatters are forced before the first gather so
     the gpsimd IRAM library is switched exactly once.
"""
import functools
import numpy as np
import ml_dtypes

B, N, C, K = 4, 4096, 128, 32
RADIUS = 0.1
BN_EPS = 1e-5
R2 = RADIUS * RADIUS
NCHUNK = 128
NCH = N // NCHUNK          # 32
QPC = N // 2               # 2048 queries per core
QT = 128                   # queries per tile
NQT = QPC // QT            # 16
QB = 512                   # query block
NQB = QPC // QB            # 4
GRP = 4                    # chunks per rank-psum group
NGRP = NCH // GRP          # 8
NCORES = 8
GC = 512                   # gather chunk (idxs per dma_gather)

bf16 = ml_dtypes.bfloat16


def _bf(x):
    return np.asarray(x, dtype=bf16).astype(np.float32)


def _split3(x):
    h = _bf(x)
    m = _bf(x - h)
    l = _bf(x - h - m)
    return h, m, l


def _build_AB(p_b, q_lo, q_hi):
    """A [24, N] (candidate side, -2 scale folded), Bm [24, Q] (query side), bf16."""
    x = p_b.astype(np.float32)
    pn = np.sum(x * x, axis=1, dtype=np.float32)
    sp = [_split3(x[:, d]) for d in range(3)]
    sp2 = [tuple(-2.0 * t for t in sp[d]) for d in range(3)]
    spn = _split3(pn)
    PAIRS = [(0, 0), (0, 1), (1, 0), (0, 2), (2, 0), (1, 1)]
    A_rows, B_rows = [], []
    for d in range(3):
        for (s, s2) in PAIRS:
            A_rows.append(sp2[d][s])
            B_rows.append(sp[d][s2][q_lo:q_hi])
    for s in range(3):
        A_rows.append(spn[s])
        B_rows.append(np.ones(q_hi - q_lo, np.float32))
    for s in range(3):
        A_rows.append(np.ones(N, np.float32))
        B_rows.append(spn[s][q_lo:q_hi])
    return (np.stack(A_rows).astype(bf16), np.stack(B_rows).astype(bf16))


def _static_tables():
    f8 = ml_dtypes.float8_e4m3fn
    # tri weights for v in {0,1}: slot contribution 32*v_t - (#valid before t)
    tri = np.zeros((NCHUNK, NCHUNK), np.float32)
    for jl in range(NCHUNK):
        tri[:jl, jl] = -1.0
        tri[jl, jl] = 32.0
    # DoubleRow moving for rank pairs: X0 = [tri | 0], X1 = [0 | tri]
    # packed two-major as [128, 512]
    tri_dr = np.zeros((NCHUNK, 2, 2 * NCHUNK), np.float32)
    tri_dr[:, 0, :NCHUNK] = tri
    tri_dr[:, 1, NCHUNK:] = tri
    tri_dr = tri_dr.reshape(NCHUNK, 4 * NCHUNK).astype(f8)
    # carry table [64, NCH*128]: rows 0..31 block-diag ones (ct3 hi), rows
    # 32..63 block-diag ones (ct3 lo); ct3 = -CNT_<c - 1.
    carry_all = np.zeros((64, NCH * NCHUNK), np.float32)
    for c in range(NCH):
        carry_all[c, c * NCHUNK:(c + 1) * NCHUNK] = 1.0
        carry_all[32 + c, c * NCHUNK:(c + 1) * NCHUNK] = 1.0
    carry_all = carry_all.astype(bf16)
    # DoubleRow one-hot stationaries for csum pairs: per pair u the two
    # halves select rows 2u and 2u+1 of the count psum
    onehot_dr = np.zeros((NCHUNK, NCH // 2, 2, NCH), np.float32)
    for u in range(NCH // 2):
        onehot_dr[:, u, 0, 2 * u] = 1.0
        onehot_dr[:, u, 1, 2 * u + 1] = 1.0
    onehot_dr = onehot_dr.reshape(NCHUNK, (NCH // 2) * 2 * NCH).astype(f8)
    iota_j = np.broadcast_to(np.arange(N, dtype=np.int16)[None, :], (128, N)).copy()
    ident = np.eye(128, dtype=np.float32)
    return tri_dr, carry_all, onehot_dr, iota_j, ident


def _kernel_body(tc, io):
    import concourse.bass as bass
    import concourse.mybir as mybir
    from contextlib import ExitStack

    nc = tc.nc
    dt = mybir.dt
    AF = mybir.ActivationFunctionType
    ALU = mybir.AluOpType

    with ExitStack() as ctx:
        const = ctx.enter_context(tc.tile_pool(name="const", bufs=1))
        w1t_sb = const.tile([128, 128], dt.bfloat16, tag="w1t")
        nc.sync.dma_start(out=w1t_sb, in_=io["w1t"])
        gamma_sb = const.tile([128, 1], dt.float32, tag="gm")
        nc.sync.dma_start(out=gamma_sb, in_=io["gamma"])
        beta_sb = const.tile([128, 1], dt.float32, tag="bt")
        nc.sync.dma_start(out=beta_sb, in_=io["beta"])
        pA_sb = const.tile([24, N], dt.bfloat16, tag="pA")
        nc.sync.dma_start(out=pA_sb, in_=io["pA"])
        pB_sb = const.tile([24, QPC], dt.bfloat16, tag="pB")
        nc.sync.dma_start(out=pB_sb, in_=io["pB"])
        tridr_sb = const.tile([128, 4 * NCHUNK], dt.float8e4, tag="tridr")
        nc.sync.dma_start(out=tridr_sb, in_=io["tri_dr"])
        tridr_v = tridr_sb.rearrange("p (two n) -> p two n", two=2)
        carry_sb = const.tile([64, NCH * 128], dt.bfloat16, tag="car")
        nc.sync.dma_start(out=carry_sb, in_=io["carry_all"])
        onehotdr_sb = const.tile([128, NCH * NCH], dt.float8e4, tag="ohdr")
        nc.sync.dma_start(out=onehotdr_sb, in_=io["onehot_dr"])
        iota_sb = const.tile([128, N], dt.int16, tag="iota")
        nc.sync.dma_start(out=iota_sb, in_=io["iota_j"])
        ident_sb = const.tile([128, 128], dt.float32, tag="idn")
        nc.sync.dma_start(out=ident_sb, in_=io["ident"])
        identb_sb = const.tile([128, 128], dt.bfloat16, tag="idb")
        nc.sync.dma_start(out=identb_sb, in_=io["ident_bf"])
        eps_t = const.tile([128, 1], dt.float32, tag="eps")
        nc.vector.memset(eps_t, BN_EPS)
        # sigmoid(-1e30*d^2 + 1e28) -> exact {0,1} validity indicator
        sgsc_t = const.tile([128, 1], dt.float32, tag="sgs")
        nc.vector.memset(sgsc_t, -1e30)
        sgbi_t = const.tile([128, 1], dt.float32, tag="sgb")
        nc.vector.memset(sgbi_t, R2 * 1e30)

        hpool = ctx.enter_context(tc.tile_pool(name="hp", bufs=1))
        h_n = hpool.tile([128, N], dt.float32, tag="hn")
        hT = hpool.tile([128, N], dt.float16, tag="hT")
        dbg_after_h = (lambda: nc.sync.dma_start(out=io["dbg_h"], in_=h_n)) \
            if "dbg_h" in io else (lambda: None)

        # ---------------- Phase A: BN stats + h_n ----------------
        ps_sq = ctx.enter_context(tc.tile_pool(name="ps_sq", bufs=2, space="PSUM"))
        with ExitStack() as actx:
            bigf = actx.enter_context(tc.tile_pool(name="bigf", bufs=1))
            ps_h = actx.enter_context(tc.tile_pool(name="ps_h", bufs=2, space="PSUM"))
            stp = actx.enter_context(tc.tile_pool(name="stp", bufs=1))

            f_sb = bigf.tile([128, B * N], dt.bfloat16, tag="f")
            nc.sync.dma_start(out=f_sb, in_=io["f_all"])
            b_idx = io["b_idx"]          # which batch this core owns

            stats = stp.tile([128, B * N // 512, 6], dt.float32, tag="st")
            for i in range(B * N // 512):
                ph = ps_h.tile([128, 512], dt.float32, tag="ph")
                nc.tensor.matmul(ph, w1t_sb, f_sb[:, i * 512:(i + 1) * 512],
                                 start=True, stop=True)
                nc.vector.bn_stats(out=stats[:, i, :], in_=ph)
            mv = stp.tile([128, 2], dt.float32, tag="mv")
            nc.vector.bn_aggr(out=mv, in_=stats)
            # rstd = 1/sqrt(var+eps); gamma2 = gamma*rstd; bias2 = beta - gamma2*mean
            sqv = stp.tile([128, 1], dt.float32, tag="sq")
            nc.scalar.activation(sqv, mv[:, 1:2], AF.Sqrt, bias=eps_t, scale=1.0)
            rstd = stp.tile([128, 1], dt.float32, tag="rs")
            nc.vector.reciprocal(rstd, sqv)
            gamma2 = stp.tile([128, 1], dt.float32, tag="g2")
            nc.vector.tensor_mul(gamma2, gamma_sb, rstd)
            gm = stp.tile([128, 1], dt.float32, tag="gmn")
            nc.vector.tensor_mul(gm, gamma2, mv[:, 0:1])
            bias2 = stp.tile([128, 1], dt.float32, tag="b2")
            nc.vector.tensor_sub(bias2, beta_sb, gm)

            for i in range(N // 512):
                ph = ps_h.tile([128, 512], dt.float32, tag="ph")
                nc.tensor.matmul(ph, w1t_sb,
                                 f_sb[:, b_idx * N + i * 512:b_idx * N + (i + 1) * 512],
                                 start=True, stop=True)
                nc.scalar.activation(h_n[:, i * 512:(i + 1) * 512], ph, AF.Relu,
                                     bias=bias2, scale=gamma2)

        dbg_after_h()

        # hT swizzled fp16 transpose in SBUF: hT[p, s*128+c] = h[c, s*128+p],
        # i.e. token n = (partition n%128, 256B stripe n//128) for dma_gather.
        with ExitStack() as tctx:
            ps_t = tctx.enter_context(tc.tile_pool(name="ps_t", bufs=2, space="PSUM"))
            for c2 in range(N // 128):
                tp2 = ps_t.tile([128, 128], dt.float32, tag="htp")
                nc.tensor.transpose(tp2, h_n[:, c2 * 128:(c2 + 1) * 128], ident_sb)
                nc.scalar.activation(hT[:, c2 * 128:(c2 + 1) * 128], tp2, AF.Copy)

        # ---------------- Phase B + C: half-split pipeline ----------------
        spool = ctx.enter_context(tc.tile_pool(name="sp", bufs=2))
        ps_cs = ctx.enter_context(tc.tile_pool(name="ps_cs", bufs=1, space="PSUM"))
        ps_tr = ctx.enter_context(tc.tile_pool(name="ps_tr", bufs=1, space="PSUM"))
        ps_rk = ctx.enter_context(tc.tile_pool(name="ps_rk", bufs=2, space="PSUM"))
        small = ctx.enter_context(tc.tile_pool(name="small", bufs=3))
        ctt_p = ctx.enter_context(tc.tile_pool(name="cttp", bufs=2))
        ipool = ctx.enter_context(tc.tile_pool(name="ip", bufs=2))
        dstp = ctx.enter_context(tc.tile_pool(name="dstp", bufs=2))
        iwpool = ctx.enter_context(tc.tile_pool(name="iw", bufs=1))
        fpool = ctx.enter_context(tc.tile_pool(name="fp", bufs=4))
        dpool = ctx.enter_context(tc.tile_pool(name="dp", bufs=4))
        opool = ctx.enter_context(tc.tile_pool(name="op", bufs=4))

        dlp_v = io["dlp_s"]          # [128, QPC*K] bf16, k reversed on host
        out_v = io["out_o"]          # [128, QPC] fp16

        zmask = small.tile([128, 256], dt.int8, tag="zm")
        nc.vector.memset(zmask, 0)

        def b_front(qb):
            """distances + per-chunk counts for one 512-query block."""
            qbs = slice(qb * QB, (qb + 1) * QB)
            s_t = []
            for c in range(NCH):
                psq = ps_sq.tile([128, QB], dt.float32, tag="sq")
                nc.tensor.matmul(psq, pA_sb[:, c * NCHUNK:(c + 1) * NCHUNK],
                                 pB_sb[:, qbs], start=True, stop=True)
                if c % 2 == 0:
                    sp2 = spool.tile([128, 2, QB], dt.float8e4, tag=f"s{c // 2}")
                    s_t.append(sp2)
                st = s_t[c // 2][:, c % 2, :]
                if c % 2 == 0:
                    nc.scalar.activation(st, psq, AF.Sigmoid, bias=sgbi_t,
                                         scale=sgsc_t)
                else:
                    nc.vector.tensor_scalar(st, psq, R2, None, op0=ALU.is_le)

            psC = ps_cs.tile([NCH, QB], dt.float32, tag="cs")
            for u in range(NCH // 2):
                nc.tensor.matmul(psC,
                                 onehotdr_sb[:, u * 2 * NCH:(u + 1) * 2 * NCH]
                                 .rearrange("p (two t) -> p two t", two=2),
                                 s_t[u], start=(u == 0), stop=(u == NCH // 2 - 1),
                                 perf_mode=mybir.MatmulPerfMode.DoubleRow)
            csT = small.tile([NCH, QB], dt.bfloat16, tag="csT")
            nc.scalar.activation(csT, psC, AF.Copy)
            csbs = []
            for qt in range(NQB):
                psc2 = ps_tr.tile([128, NCH], dt.bfloat16, tag="trp")
                nc.tensor.transpose(psc2, csT[:, qt * QT:(qt + 1) * QT],
                                    identb_sb[0:NCH, 0:NCH])
                csb = small.tile([128, NCH], dt.float32, tag=f"csb{qt}")
                nc.vector.tensor_copy(csb, psc2)
                csbs.append(csb)
            return s_t, csbs

        def b_rank(qb, qt, s_t, csbs):
            """slot ids for one 128-query tile -> idxs_sc [q, N] int16."""
            ti = qb * NQB + qt
            qs = slice(qt * QT, (qt + 1) * QT)
            csb = csbs[qt]
            pref = small.tile([128, NCH], dt.float32, tag="pf0")
            nc.vector.memset(pref[:, 0:1], 0.0)
            nc.vector.tensor_copy(pref[:, 1:], csb[:, :NCH - 1])
            for sh in (1, 2, 4, 8, 16):
                pref2 = small.tile([128, NCH], dt.float32, tag=f"pf{sh}")
                nc.vector.tensor_copy(pref2[:, :sh], pref[:, :sh])
                nc.vector.tensor_add(pref2[:, sh:], pref[:, sh:], pref[:, :NCH - sh])
                pref = pref2
            ct3 = small.tile([128, NCH], dt.float32, tag="ct3")
            nc.vector.tensor_scalar(ct3, pref, -1.0, -1.0,
                                    op0=ALU.mult, op1=ALU.add)
            ctpack = small.tile([128, 2 * NCH], dt.bfloat16, tag="ctp")
            nc.vector.tensor_copy(ctpack[:, :NCH], ct3)
            nc.vector.tensor_sub(ctpack[:, NCH:], ct3, ctpack[:, :NCH])
            psP = ps_tr.tile([2 * NCH, 128], dt.bfloat16, tag="ctpT")
            nc.tensor.transpose(psP, ctpack, identb_sb)
            ctt64 = ctt_p.tile([2 * NCH, 128], dt.bfloat16, tag="ctt")
            nc.scalar.activation(ctt64, psP, AF.Copy)

            idxs_sc = ipool.tile([128, N], dt.int16, tag="isc")
            for g in range(NGRP):
                pr = ps_rk.tile([128, GRP * 128], dt.float32, tag="rk")
                nc.tensor.matmul(pr, ctt64,
                                 carry_sb[:, g * GRP * 128:(g + 1) * GRP * 128],
                                 start=True, stop=False)
                for up in range(GRP // 2):
                    u = g * (GRP // 2) + up
                    nc.tensor.matmul(pr[:, up * 256:(up + 1) * 256],
                                     s_t[u][:, :, qs], tridr_v,
                                     start=False, stop=(up == GRP // 2 - 1),
                                     skip_group_check=(up != GRP // 2 - 1),
                                     perf_mode=mybir.MatmulPerfMode.DoubleRow)
                nc.scalar.activation(idxs_sc[:, g * GRP * 128:(g + 1) * GRP * 128],
                                     pr, AF.Copy)
            if "dbg_slots" in io and ti == 0:
                nc.sync.dma_start(out=io["dbg_slots"], in_=idxs_sc)
            return idxs_sc

        def b_scatter(ti, idxs_sc):
            """first-K extraction + wrapped idx layout for one tile."""
            dst = dstp.tile([128, K], dt.int16, tag="dst")
            nc.gpsimd.local_scatter(dst, iota_sb, idxs_sc,
                                    channels=128, num_elems=K, num_idxs=N)
            mask = small.tile([128, K], dt.int8, tag="msk")
            nc.vector.tensor_scalar(mask, dst, 0.0, None, op0=ALU.is_equal)
            nc.vector.copy_predicated(dst, mask, dst[:, K - 1:K].to_broadcast((128, K)))
            if "dbg_dst" in io:
                nc.sync.dma_start(out=io["dbg_dst"][:, ti * K:(ti + 1) * K], in_=dst)
            dstf = small.tile([128, 256], dt.float32, tag="dstf")
            dfv = dstf.rearrange("p (h r s) -> p h r s", h=2, r=8)
            dst_b = bass.AP(tensor=dst.tensor, offset=dst.offset,
                            ap=[dst.ap[0], [16, 2], [0, 8], [1, 16]])
            nc.vector.tensor_copy(dfv, dst_b)
            idxw = iwpool.tile([128, 256], dt.int16, tag=f"idxw{ti}")
            iwv = idxw.rearrange("p (c two) -> p c two", two=2)
            for half in range(2):
                tps = ps_tr.tile([128, 128], dt.float32, tag="tp")
                nc.tensor.transpose(tps, dstf[:, half * 128:(half + 1) * 128],
                                    ident_sb)
                nc.scalar.activation(iwv[:, :, half], tps, AF.Copy)
            return idxw, dst

        def c_order(tiles):
            """force this half's gathers after its last local_scatter."""
            last_dst = tiles[-1][1]
            for idxw, _ in tiles:
                nc.vector.copy_predicated(idxw, zmask,
                                          last_dst[:, 0:1].to_broadcast((128, 256)))

        def c_tile(ti, idxw):
            """gather + add + max-pool for one tile."""
            fj = fpool.tile([128, QT * K], dt.float16, tag="fj")
            fj_v = fj.rearrange("p (o i) -> p o i", o=1)
            for gc in range(QT * K // GC):
                nc.gpsimd.dma_gather(
                    fj_v[:, :, gc * GC:(gc + 1) * GC], hT,
                    idxw[:, gc * (GC // 16):(gc + 1) * (GC // 16)],
                    num_idxs=GC, num_idxs_reg=GC, elem_size=128,
                    transpose=True,
                    sbuf_tokens_per_rank=128,
                    sbuf_free_dim_per_rank=256,
                    queue_num=(ti * (QT * K // GC) + gc) % 2)
            if "dbg_fj" in io and ti == 0:
                nc.sync.dma_start(out=io["dbg_fj"], in_=fj)
            dlp_t = dpool.tile([128, QT * K], dt.bfloat16, tag="dl")
            nc.sync.dma_start(out=dlp_t,
                              in_=dlp_v[:, ti * QT * K:(ti + 1) * QT * K])
            nc.vector.tensor_add(fj, fj, dlp_t)
            out_t = opool.tile([128, QT], dt.float16, tag="ot")
            nc.vector.tensor_reduce(out_t,
                                    fj.rearrange("p (q k) -> p q k", k=K),
                                    axis=mybir.AxisListType.X, op=ALU.max)
            nc.sync.dma_start(out=out_v[:, ti * QT:(ti + 1) * QT], in_=out_t)

        # ---- phase B for all tiles (scatters on gpsimd lib 7) ----
        tiles = []
        for qb in range(NQB):
            s_t, csbs = b_front(qb)
            for qt in range(NQB):
                ti = qb * NQB + qt
                tiles.append(b_scatter(ti, b_rank(qb, qt, s_t, csbs)))

        # ---- phase C for all tiles (gathers on gpsimd lib 3) ----
        c_order(tiles)
        for ti, (idxw, _) in enumerate(tiles):
            c_tile(ti, idxw)


@functools.lru_cache(maxsize=1)
def _compiled():
    import concourse.bacc as bacc
    import concourse.tile as tile
    import concourse.mybir as mybir

    dt = mybir.dt
    nc = bacc.Bacc("TRN2", target_bir_lowering=False, debug=False,
                   num_devices=NCORES, num_swdge_queues=2)
    io = {}

    def din(name, shape, dtype):
        io[name] = nc.dram_tensor(name, shape, dtype, kind="ExternalInput").ap()

    din("f_all", [128, B * N], dt.bfloat16)
    din("w1t", [128, 128], dt.bfloat16)
    din("gamma", [128, 1], dt.float32)
    din("beta", [128, 1], dt.float32)
    din("pA", [24, N], dt.bfloat16)
    din("pB", [24, QPC], dt.bfloat16)
    din("tri_dr", [128, 4 * NCHUNK], dt.float8e4)
    din("carry_all", [64, NCH * 128], dt.bfloat16)
    din("onehot_dr", [128, NCH * NCH], dt.float8e4)
    din("iota_j", [128, N], dt.int16)
    din("ident", [128, 128], dt.float32)
    din("ident_bf", [128, 128], dt.bfloat16)
    din("dlp_s", [128, QPC * K], dt.bfloat16)
    io["out_o"] = nc.dram_tensor("out_o", [128, QPC], dt.float16,
                                 kind="ExternalOutput").ap()
    import os
    if os.environ.get("KM_DEBUG"):
        io["dbg_h"] = nc.dram_tensor("dbg_h", [128, N], dt.float32,
                                     kind="ExternalOutput").ap()
        io["dbg_slots"] = nc.dram_tensor("dbg_slots", [128, N], dt.int16,
                                         kind="ExternalOutput").ap()
        io["dbg_dst"] = nc.dram_tensor("dbg_dst", [128, NQT * K], dt.int16,
                                       kind="ExternalOutput").ap()
        io["dbg_fj"] = nc.dram_tensor("dbg_fj", [128, QT * K], dt.float16,
                                      kind="ExternalOutput").ap()

    # All cores run the same program; the host rotates f_all so each core's
    # own batch occupies columns [0, N), hence b_idx is constant 0.
    io["b_idx"] = 0

    with tile.TileContext(nc) as tc:
        _kernel_body(tc, io)
    nc.compile()
    return nc


def _host_prep(inputs):
    p = np.asarray(inputs["p"], np.float32)
    f = np.asarray(inputs["f"], np.float32)
    dlp = np.asarray(inputs["dlp"], np.float32)
    W1 = np.asarray(inputs["W1"], np.float32)
    gamma = np.asarray(inputs["gamma"], np.float32)
    beta = np.asarray(inputs["beta"], np.float32)

    tri_dr, carry_all, onehot_dr, iota_j, ident = _static_tables()
    f_all = np.ascontiguousarray(np.moveaxis(f, 0, 1).reshape(C, B * N))
    w1t = np.ascontiguousarray(W1.T).astype(bf16)

    in_maps = []
    for core in range(NCORES):
        b, half = core // 2, core % 2
        q_lo, q_hi = half * QPC, (half + 1) * QPC
        pA, pB = _build_AB(p[b], q_lo, q_hi)
        dlp_s = np.ascontiguousarray(
            dlp[b, :, q_lo:q_hi, ::-1].reshape(C, QPC * K)).astype(bf16)
        # rotate f so this core's batch occupies columns [0, N)
        f_rot = np.ascontiguousarray(
            np.concatenate([f_all[:, b * N:], f_all[:, :b * N]], axis=1)
        ).astype(bf16)
        in_maps.append({
            "f_all": f_rot,
            "w1t": w1t,
            "gamma": np.ascontiguousarray(gamma.reshape(C, 1)),
            "beta": np.ascontiguousarray(beta.reshape(C, 1)),
            "pA": pA, "pB": pB,
            "tri_dr": tri_dr, "carry_all": carry_all,
            "onehot_dr": onehot_dr, "iota_j": iota_j, "ident": ident,
            "ident_bf": ident.astype(bf16),
            "dlp_s": dlp_s,
        })
    return in_maps


def run(inputs, trace=False, **kw):
    from concourse.bass_utils import run_bass_kernel_spmd
    nc = _compiled()
    in_maps = _host_prep(inputs)
    res = run_bass_kernel_spmd(nc, in_maps, core_ids=list(range(NCORES)),
                               trace=trace, **kw)
    out = np.zeros((B, C, N), np.float32)
    for core in range(NCORES):
        b, half = core // 2, core % 2
        out[b, :, half * QPC:(half + 1) * QPC] = \
            res.results[core]["out_o"].astype(np.float32)
    return out, res


def kernel(**inputs) -> np.ndarray:
    out, _ = run(inputs, trace=False)
    return out
